# revision 5
# baseline (speedup 1.0000x reference)
"""XNOR-linear (sign-quantized GEMM) kernel for 8 Trainium2 NeuronCores.

Reference computation:
    out = (sign(x) @ sign(w).T) * mean(|w|) * mean(|x|) + bias
with x [N, CIN] fp32, w [COUT, CIN] fp32, bias [COUT] fp32.

Strategy (data-parallel over the batch dim):
  - Host: cast x/w to bf16 (sign-preserving), transpose to contraction-major
    layouts, shard x rows across 8 cores, replicate w/bias.
  - Device (per core, Tile framework):
      * quantize xT/wT tiles to fp8e4 {-1,0,+1} on ScalarE (Sign activation)
      * abs-sum reductions on VectorE for the two means
      * 512-byte AllReduce for the global sum(|x|) across cores
      * fp8 DoubleRow matmuls (K=256 per instruction) accumulating exact
        integer dot products in fp32 PSUM
      * fused (psum * alpha*betta + bias) on PSUM eviction (VectorE)
  - Output per core: outT [COUT, n_shard] fp32; host transposes + concats.
"""

from contextlib import ExitStack

import numpy as np
import ml_dtypes

import concourse.bass as bass
import concourse.tile as tile
from concourse import bacc, mybir
from concourse.bass_utils import run_bass_kernel_spmd

P = 128
FD = 512  # matmul moving free dim (one fp32 PSUM bank)


def build_nc(n_total, cin, cout, n_cores):
    """Build the per-core Bass program (SPMD: same program on every core)."""
    n_shard = n_total // n_cores
    KT = cin // P        # contraction tiles of 128
    KP = KT // 2         # DoubleRow pairs (K=256 each)
    OT = cout // P       # output-channel tiles (psum partition dim)
    NH = (n_shard + FD - 1) // FD  # moving-dim chunks
    assert cin % 256 == 0 and cout % P == 0 and n_total % n_cores == 0

    fp32 = mybir.dt.float32
    bf16 = mybir.dt.bfloat16
    fp8 = mybir.dt.float8e4

    nc = bacc.Bacc(
        "TRN2", target_bir_lowering=False, debug=False, num_devices=n_cores
    )

    xT = nc.dram_tensor("xT", [cin, n_shard], bf16, kind="ExternalInput")
    wT = nc.dram_tensor("wT", [cin, cout], bf16, kind="ExternalInput")
    biasT = nc.dram_tensor("biasT", [P, OT], fp32, kind="ExternalInput")
    outT = nc.dram_tensor("outT", [cout, n_shard], fp32, kind="ExternalOutput")

    ccin = nc.dram_tensor("ccin", [P, 1], fp32)
    ccout = nc.dram_tensor("ccout", [P, 1], fp32)

    # alpha*betta = (sum|w| / (cout*cin)) * (sum|x| / (n_total*cin))
    ab_scale = 1.0 / (float(cout) * float(cin) * float(n_total) * float(cin))

    with tile.TileContext(nc) as tc, ExitStack() as ctx:
        const = ctx.enter_context(tc.tile_pool(name="const", bufs=1))
        qx_pool = ctx.enter_context(tc.tile_pool(name="qx", bufs=1))
        qw_pool = ctx.enter_context(tc.tile_pool(name="qw", bufs=1))
        xin_pool = ctx.enter_context(tc.tile_pool(name="xin", bufs=3))
        win_pool = ctx.enter_context(tc.tile_pool(name="win", bufs=2))
        out_pool = ctx.enter_context(tc.tile_pool(name="outsb", bufs=4))
        psum_pool = ctx.enter_context(tc.tile_pool(name="psum", bufs=7, space="PSUM"))
        psum_red = ctx.enter_context(tc.tile_pool(name="psumred", bufs=1, space="PSUM"))

        ones_t = const.tile([P, P], fp32, name="ones")
        nc.vector.memset(ones_t[:], 1.0)
        bias_sb = const.tile([P, OT], fp32, name="bias_sb")
        nc.sync.dma_start(out=bias_sb[:], in_=biasT[:, :])
        xacc = const.tile([P, KT], fp32, name="xacc")
        wacc = const.tile([P, KT], fp32, name="wacc")
        xacc2 = const.tile([P, 1], fp32, name="xacc2")
        red_in = const.tile([P, 2], fp32, name="red_in")
        red_sb = const.tile([P, 2], fp32, name="red_sb")
        ab = const.tile([P, 1], fp32, name="ab")

        qx = [qx_pool.tile([P, 2, n_shard], fp8, name=f"qx{i}") for i in range(KP)]
        qw = [qw_pool.tile([P, 2, cout], fp8, name=f"qw{i}") for i in range(KP)]

        # ---- Phase X: load + quantize x shard, per-partition |x| sums ----
        for kt in range(KT):
            xin = xin_pool.tile([P, n_shard], bf16, name="xin")
            nc.sync.dma_start(out=xin[:], in_=xT[kt * P:(kt + 1) * P, :])
            nc.scalar.sign(qx[kt // 2][:, kt % 2, :], xin[:])
            nc.vector.tensor_reduce(
                out=xacc[:, kt:kt + 1], in_=xin[:],
                axis=mybir.AxisListType.X, op=mybir.AluOpType.add,
                apply_absolute_value=True,
            )

        # ---- Global sum(|x|): per-partition partials -> AllReduce ----
        nc.vector.tensor_reduce(
            out=xacc2[:], in_=xacc[:],
            axis=mybir.AxisListType.X, op=mybir.AluOpType.add,
        )
        nc.sync.dma_start(out=ccin[:, :], in_=xacc2[:])
        nc.gpsimd.collective_compute(
            "AllReduce",
            mybir.AluOpType.add,
            replica_groups=[list(range(n_cores))],
            ins=[ccin.ap().opt()],
            outs=[ccout.ap().opt()],
        )
        nc.sync.dma_start(out=red_in[:, 0:1], in_=ccout[:, :])

        # ---- Phase W: load + quantize weights, |w| sums ----
        for kt in range(KT):
            win = win_pool.tile([P, cout], bf16, name="win")
            nc.sync.dma_start(out=win[:], in_=wT[kt * P:(kt + 1) * P, :])
            nc.scalar.sign(qw[kt // 2][:, kt % 2, :], win[:])
            nc.vector.tensor_reduce(
                out=wacc[:, kt:kt + 1], in_=win[:],
                axis=mybir.AxisListType.X, op=mybir.AluOpType.add,
                apply_absolute_value=True,
            )
        nc.vector.tensor_reduce(
            out=red_in[:, 1:2], in_=wacc[:],
            axis=mybir.AxisListType.X, op=mybir.AluOpType.add,
        )

        # ---- Cross-partition sums via ones-matmul; fold into ab scalar ----
        # psum_r[p, 0] = sum_k ccout[k]  (global sum|x|, same in all p)
        # psum_r[p, 1] = sum_k wacc2[k]  (sum|w|)
        psum_r = psum_red.tile([P, 2], fp32, name="psr")
        nc.tensor.matmul(psum_r[:], lhsT=ones_t[:], rhs=red_in[:], start=True,
                         stop=True)
        nc.vector.tensor_copy(red_sb[:], psum_r[:])
        nc.vector.tensor_scalar(
            out=ab[:], in0=red_sb[:, 0:1],
            scalar1=red_sb[:, 1:2], scalar2=float(ab_scale),
            op0=mybir.AluOpType.mult, op1=mybir.AluOpType.mult,
        )

        # ---- Main: fp8 DoubleRow matmuls + fused scale/bias eviction ----
        for ot in range(OT):
            for nh in range(NH):
                n0 = nh * FD
                nf = min(FD, n_shard - n0)
                psum_t = psum_pool.tile([P, FD], fp32, name="ps")
                for kp in range(KP):
                    nc.tensor.matmul(
                        psum_t[:, :nf],
                        lhsT=qw[kp][:, :, ot * P:(ot + 1) * P],
                        rhs=qx[kp][:, :, n0:n0 + nf],
                        start=(kp == 0),
                        stop=(kp == KP - 1),
                        perf_mode=mybir.MatmulPerfMode.DoubleRow,
                    )
                osb = out_pool.tile([P, FD], fp32, name="osb")
                nc.vector.tensor_scalar(
                    out=osb[:, :nf], in0=psum_t[:, :nf],
                    scalar1=ab[:, 0:1], scalar2=bias_sb[:, ot:ot + 1],
                    op0=mybir.AluOpType.mult, op1=mybir.AluOpType.add,
                )
                nc.sync.dma_start(
                    out=outT[ot * P:(ot + 1) * P, n0:n0 + nf], in_=osb[:, :nf]
                )

    nc.compile()
    return nc


_CACHE = {}


def _get_nc(n_total, cin, cout, n_cores):
    key = (n_total, cin, cout, n_cores)
    if key not in _CACHE:
        _CACHE[key] = build_nc(n_total, cin, cout, n_cores)
    return _CACHE[key]


def prep_inputs(input_x, weight, bias, n_cores=8):
    """Host-side sharding: bf16 casts + transposes. Returns in_maps."""
    bf16 = ml_dtypes.bfloat16
    n_total, cin = input_x.shape
    cout = weight.shape[0]
    n_shard = n_total // n_cores
    wT = weight.T.astype(bf16, order="C")               # [cin, cout]
    biasT = np.asarray(bias, np.float32).reshape(cout // P, P).T.copy()  # [P, OT]
    in_maps = []
    for c in range(n_cores):
        xs = input_x[c * n_shard:(c + 1) * n_shard, :]
        xTs = xs.T.astype(bf16, order="C")              # [cin, n_shard]
        in_maps.append({"xT": xTs, "wT": wT, "biasT": biasT})
    return in_maps


def gather_output(results, n_total, cout, n_cores=8):
    n_shard = n_total // n_cores
    out = np.empty((n_total, cout), np.float32)
    for c in range(n_cores):
        out[c * n_shard:(c + 1) * n_shard, :] = results[c]["outT"].T
    return out


def run(input_x, weight, bias, n_cores=8, trace=False):
    n_total, cin = input_x.shape
    cout = weight.shape[0]
    nc = _get_nc(n_total, cin, cout, n_cores)
    in_maps = prep_inputs(input_x, weight, bias, n_cores)
    res = run_bass_kernel_spmd(nc, in_maps, list(range(n_cores)), trace=trace)
    out = gather_output(res.results, n_total, cout, n_cores)
    return out, res


def kernel(input_x, weight, bias):
    out, _ = run(
        np.asarray(input_x, np.float32),
        np.asarray(weight, np.float32),
        np.asarray(bias, np.float32),
        n_cores=8,
    )
    return out
